# revision 4
# baseline (speedup 1.0000x reference)
"""Trainium2 Bass kernel v3 for char-CNN (embed lookup + conv1d(K=5,pad=2) + bias + maxpool).

Math: out[n, f] = b[f] + max_w sum_k G_k[ids[n, w+k-2], f]
where G_k[v, f] = sum_d E[v, d] * Wc[f, d, k] (host-folded, weights only).

v3 vs v2 (344us):
  * ids broadcast to 96 partitions by DMA (stride-0 partition read from
    DRAM, int32->f32 convert) instead of contraction-1 PE matmuls: the PE
    now runs ONLY tap matmuls.
  * ONE one-hot per unit (fp8e5, hit value 2^-13) feeds both the f32r hi
    matmuls (tables pre-scaled by 2^13 -- exact power-of-2) and the fp8
    DoubleRow lo matmuls (tables are (G-hi)*2^13 e4m3). Halves is_equal
    work on the DVE.
  * PSUM entirely for accumulators: psY bufs=4 x 2 tags = 8 banks.
Per unit-half: 5 hi f32r matmuls (512 cols) + 3 DR matmuls (2 taps each).
"""

import numpy as np

import concourse.bass as bass
import concourse.bacc as bacc
import concourse.mybir as mybir
from concourse.ap import AP
from concourse.tile import TileContext
from concourse.bass_utils import run_bass_kernel_spmd

N, W = 32768, 16
VOCAB, D, F, K = 96, 100, 100, 5
N_CORES = 8
NSH = N // N_CORES            # 4096 tokens/core
UNIT = 64                     # tokens per unit
NUNIT = NSH // UNIT           # 64
GROUP = 512                   # tokens per output block
NGROUP = NSH // GROUP         # 8
UPG = GROUP // UNIT           # 8
WP = W + 4                    # padded char positions (20)
HT = UNIT // 2                # tokens per half (32)

import os as _os
SINGLE_HOT = _os.environ.get("KV3_SINGLE_HOT", "0") == "1"
PSY_BUFS = int(_os.environ.get("KV3_PSY_BUFS", "4"))
WARM_MM = _os.environ.get("KV3_WARM", "1") == "1"
LO_SCALE = 13
DR_PAIRS = [(0, (0, 1)), (2, (2, 3)), (3, (None, 4))]

f16 = mybir.dt.float16
f32 = mybir.dt.float32
f32r = mybir.dt.float32r
f8e4 = mybir.dt.float8e4
f8e5 = mybir.dt.float8e5
i32 = mybir.dt.int32


def build_nc():
    nc = bacc.Bacc("TRN2", target_bir_lowering=False)

    ids_d = nc.dram_tensor("ids", [NSH, W], i32, kind="ExternalInput")
    gthi_d = nc.dram_tensor("gthi", [VOCAB, K, F], f16 if SINGLE_HOT else f32r, kind="ExternalInput")
    gtlo_d = nc.dram_tensor("gtlo", [VOCAB, 3, 2, 128], f8e4, kind="ExternalInput")
    iota_d = nc.dram_tensor("iota", [VOCAB, 1], f32, kind="ExternalInput")
    bias_d = nc.dram_tensor("bias", [F, 1], f32, kind="ExternalInput")
    out_d = nc.dram_tensor("out", [NGROUP, F, GROUP], f32, kind="ExternalOutput")

    dma_engines = None  # engines whose queues carry the ids broadcast DMAs

    with TileContext(nc) as tc:
        with (
            tc.tile_pool(name="consts", bufs=1) as consts,
            tc.tile_pool(name="outp", bufs=2) as outp,
            tc.tile_pool(name="bip", bufs=3) as bip,
            tc.tile_pool(name="psY", bufs=PSY_BUFS, space="PSUM") as psY,
        ):
            dma_engines = [nc.gpsimd, nc.gpsimd, nc.gpsimd]
            iota_t = consts.tile([VOCAB, 1], f32)
            nc.scalar.dma_start(out=iota_t, in_=iota_d[:, :])
            dve_warm = consts.tile([VOCAB, 1], f32, tag="dve_warm")
            nc.vector.tensor_scalar(
                out=dve_warm[:, :], in0=iota_t[:, :],
                scalar1=iota_t[:, 0:1], scalar2=None,
                op0=mybir.AluOpType.is_equal,
            )
            gthi = consts.tile([VOCAB, K, F], f16 if SINGLE_HOT else f32r)
            nc.sync.dma_start(
                out=gthi.rearrange("v k f -> v (k f)"),
                in_=gthi_d.rearrange("v k f -> v (k f)"),
            )
            gtlo = consts.tile([VOCAB, 3, 2, 128], f8e4)
            nc.sync.dma_start(
                out=gtlo.rearrange("v p j f -> v (p j f)"),
                in_=gtlo_d.rearrange("v p j f -> v (p j f)"),
            )
            bias_t = consts.tile([F, 1], f32)
            nc.scalar.dma_start(out=bias_t, in_=bias_d[:, :])

            ol_tiles, oh_tiles = [], []
            for j in range(2):
                ol = consts.tile([VOCAB, UNIT, WP], f8e5, tag=f"ol{j}")
                nc.vector.memset(
                    ol.rearrange("v t w -> v (t w)").bitcast(mybir.dt.uint8), 0
                )
                ol_tiles.append(ol)
                if not SINGLE_HOT:
                    oh = consts.tile([VOCAB, UNIT, WP], f32r, tag=f"oh{j}")
                    nc.vector.memset(
                        oh.rearrange("v t w -> v (t w)").bitcast(f32), 0.0
                    )
                    oh_tiles.append(oh)

            # PE warmup against HAM throttle during init DMAs
            warm = psY.tile([1, 1], f32, tag="y0")
            for _ in range(48 if WARM_MM else 0):
                nc.tensor.matmul(
                    warm[0:1, 0:1], iota_t[0:1, 0:1], iota_t[0:1, 0:1],
                    start=True, stop=True,
                )

            bi_tiles = {}

            def load_bi(g):
                # ids for the group, broadcast across 96 partitions by DMA
                # (stride-0 partition read), int32 -> f32 convert, one DMA
                # per unit rotated over 4 queues.
                bi = bip.tile([VOCAB, GROUP * W], i32, tag="bi")
                for uu in range(UPG):
                    src = ids_d[
                        g * GROUP + uu * UNIT : g * GROUP + (uu + 1) * UNIT, :
                    ].rearrange("(a t) w -> a (t w)", a=1).partition_broadcast(VOCAB)
                    dma_engines[uu % 3].dma_start(
                        out=bi[:, uu * UNIT * W : (uu + 1) * UNIT * W],
                        in_=src,
                    )
                bi_tiles[g] = bi

            def onehot(u):
                g, uu = divmod(u, UPG)
                bi = bi_tiles[g]
                ol = ol_tiles[u % 2]
                nc.vector.tensor_scalar(
                    out=ol[0:VOCAB, :, 2 : 2 + W],
                    in0=bi[:, uu * UNIT * W : (uu + 1) * UNIT * W].rearrange(
                        "v (t w) -> v t w", t=UNIT
                    ),
                    scalar1=iota_t[:, 0:1], scalar2=float(2.0 ** -LO_SCALE),
                    op0=mybir.AluOpType.is_equal,
                    op1=mybir.AluOpType.mult,
                )
                if not SINGLE_HOT:
                    oh = oh_tiles[u % 2]
                    nc.vector.tensor_scalar(
                        out=oh[0:VOCAB, :, 2 : 2 + W],
                        in0=bi[:, uu * UNIT * W : (uu + 1) * UNIT * W].rearrange(
                            "v (t w) -> v t w", t=UNIT
                        ),
                        scalar1=iota_t[:, 0:1], scalar2=None,
                        op0=mybir.AluOpType.is_equal,
                    )

            def dr_rhs(ol, h, base):
                b = ol[0:VOCAB, h * HT : (h + 1) * HT, base : base + W + 1]
                ap = [list(b.ap[0]), [1, 2], list(b.ap[1]), [list(b.ap[2])[0], W]]
                return AP(b.tensor, b.offset, ap)

            def taps(u, out_sb):
                g, uu = divmod(u, UPG)
                ol = ol_tiles[u % 2]
                oh = ol if SINGLE_HOT else oh_tiles[u % 2]
                for h in range(2):
                    ys = psY.tile([F, HT, W], f32, tag=f"y{h}", name=f"y{h}_{u}")
                    t0 = h * HT
                    order = [("hi", 0), ("dr", 0), ("hi", 1), ("dr", 1),
                             ("hi", 2), ("dr", 2), ("hi", 3), ("hi", 4)]
                    last = len(order) - 1
                    for i, (kind, k) in enumerate(order):
                        if kind == "hi":
                            nc.tensor.matmul(
                                ys[:, :, :],
                                gthi[:, k, :],
                                oh[0:VOCAB, t0 : t0 + HT, k : k + W],
                                start=(i == 0), stop=(i == last),
                                skip_group_check=True,
                            )
                        else:
                            base = DR_PAIRS[k][0]
                            nc.tensor.matmul(
                                ys[:, :, :],
                                gtlo[:, k, :, 0:F],
                                dr_rhs(ol, h, base),
                                start=(i == 0), stop=(i == last),
                                perf_mode=mybir.MatmulPerfMode.DoubleRow,
                                skip_group_check=True,
                            )
                    nc.vector.reduce_max(
                        out=out_sb[:, uu * UNIT + t0 : uu * UNIT + t0 + HT],
                        in_=ys[:, :, :],
                        axis=mybir.AxisListType.X,
                    )

            load_bi(0)
            load_bi(1)
            onehot(0)
            out_sb = None
            for u in range(NUNIT):
                g, uu = divmod(u, UPG)
                if uu == 0:
                    out_sb = outp.tile([F, GROUP], f32, tag="osb")
                    if g + 2 < NGROUP:
                        load_bi(g + 2)
                if u + 1 < NUNIT:
                    onehot(u + 1)
                taps(u, out_sb)
                if uu == UPG // 2 - 1 or uu == UPG - 1:
                    c0 = 0 if uu < UPG // 2 else GROUP // 2
                    c1 = c0 + GROUP // 2
                    nc.any.tensor_scalar(
                        out=out_sb[:, c0:c1], in0=out_sb[:, c0:c1],
                        scalar1=bias_t[:, 0:1], scalar2=None,
                        op0=mybir.AluOpType.add,
                    )
                    nc.sync.dma_start(out=out_d[g, :, c0:c1], in_=out_sb[:, c0:c1])

    nc.compile()
    return nc


def _round_f32r(x):
    b = np.asarray(x, np.float32).view(np.uint32)
    b = (b + 0x800) & np.uint32(0xFFFFF000)
    return b.view(np.float32)


def make_consts(embed_table, conv_w, conv_b):
    G = np.einsum(
        "vd,fdk->kvf", embed_table.astype(np.float64), conv_w.astype(np.float64)
    )  # [K, 96, F]
    if SINGLE_HOT:
        # hi in fp16 (walrus forbids mixing 32-bit f32r with the fp8 one-hot);
        # tables carry 2^13 so the 2^-13-valued one-hot cancels it exactly
        hi = G.astype(np.float16).astype(np.float64)
        gthi = np.ascontiguousarray(
            (np.transpose(hi, (1, 0, 2)) * float(2 ** LO_SCALE)).astype(np.float16)
        )
    else:
        hi = _round_f32r(G.astype(np.float32)).astype(np.float64)
        gthi = np.ascontiguousarray(np.transpose(hi, (1, 0, 2)).astype(np.float32))
    f8np = mybir.dt.np(f8e4)
    lo = ((G - hi) * float(2 ** LO_SCALE)).astype(np.float32)
    lo8 = lo.astype(f8np)
    gtlo = np.zeros((VOCAB, 3, 2, 128), f8np)
    for p, (base, (j0, j1)) in enumerate(DR_PAIRS):
        if j0 is not None:
            gtlo[:, p, 0, 0:F] = lo8[j0]
        if j1 is not None:
            gtlo[:, p, 1, 0:F] = lo8[j1]
    iota = np.arange(VOCAB, dtype=np.float32).reshape(VOCAB, 1)
    bias = conv_b.astype(np.float32).reshape(F, 1)
    return gthi, gtlo, iota, bias


_NC_CACHE = {}
TRACE = False
LAST_RESULT = None


def kernel(char_ids, embed_table, conv_w, conv_b):
    global LAST_RESULT
    char_ids = np.asarray(char_ids)
    gthi, gtlo, iota, bias = make_consts(
        np.asarray(embed_table), np.asarray(conv_w), np.asarray(conv_b)
    )

    if "nc" not in _NC_CACHE:
        _NC_CACHE["nc"] = build_nc()
    nc = _NC_CACHE["nc"]

    in_maps = []
    for c in range(N_CORES):
        shard = np.ascontiguousarray(char_ids[c * NSH : (c + 1) * NSH])
        in_maps.append(
            {"ids": shard, "gthi": gthi, "gtlo": gtlo, "iota": iota, "bias": bias}
        )

    kwargs = {}
    if TRACE:
        kwargs = dict(trace=True, trace_cores=list(range(N_CORES)))
    res = run_bass_kernel_spmd(nc, in_maps, core_ids=list(range(N_CORES)), **kwargs)
    LAST_RESULT = res

    out = np.empty((N, F), np.float32)
    for c in range(N_CORES):
        o = res.results[c]["out"]  # [NGROUP, F, GROUP]
        out[c * NSH : (c + 1) * NSH] = o.transpose(0, 2, 1).reshape(NSH, F)
    return out
